# revision 1
# baseline (speedup 1.0000x reference)
"""Cross-attention kernel for Trainium2, 8 NeuronCores.

Problem: B=2, T=S=2048, DM=1024, H=16, HD=64, partial RoPE on first 32 dims.
Sharding: batch (2-way) x head-group (4-way, 4 heads each) = 8 cores.
Each core computes its head-group's contribution to the full output
(out_partial = attn_heads @ Wo_rows); host sums 4 partials per batch.

Layouts (per core):
  qT/kT  [256, 2048]  head-dim on partitions (2 tiles of 128 = head pairs)
  V      [2048, 4*65] s on partitions, per head 64 cols + ones col (softmax sums)
  scoresT[s, t] from PE; exp on ScalarE (scale=1/8, no max-subtract: logits
  are small by construction); attn@V accumulates [65, t] in PSUM where row 64
  = sumexp; normalization applied after AV (exp is unnormalized softmax).
"""

import numpy as np

B, T, S, DM = 2, 2048, 2048, 1024
H, HD, N_ELEM = 16, 64, 32
HG = 4          # heads per core
NCORES = 8

_cached = {}


def _build_program():
    import concourse.bass as bass
    import concourse.tile as tile
    from concourse import bacc, mybir
    from concourse.bass import ts, ds

    f32 = mybir.dt.float32
    bf16 = mybir.dt.bfloat16
    Exp = mybir.ActivationFunctionType.Exp

    nc = bacc.Bacc(
        "TRN2",
        target_bir_lowering=False,
        debug=False,
        enable_asserts=False,
        num_devices=NCORES,
    )

    xT_d = nc.dram_tensor("xT", [DM, T], bf16, kind="ExternalInput").ap()
    yT_d = nc.dram_tensor("yT", [DM, S], bf16, kind="ExternalInput").ap()
    wq_d = nc.dram_tensor("wq", [DM, 256], bf16, kind="ExternalInput").ap()
    wk_d = nc.dram_tensor("wk", [DM, 256], bf16, kind="ExternalInput").ap()
    wv_d = nc.dram_tensor("wv", [DM, 256], bf16, kind="ExternalInput").ap()
    wo_d = nc.dram_tensor("wo", [256, DM], bf16, kind="ExternalInput").ap()
    cext_d = nc.dram_tensor("cext", [128, T], f32, kind="ExternalInput").ap()
    sext_d = nc.dram_tensor("sext", [128, T], f32, kind="ExternalInput").ap()
    out_d = nc.dram_tensor("out", [T, DM], f32, kind="ExternalOutput").ap()

    with tile.TileContext(nc) as tc:
        with tc.tile_pool(name="const", bufs=1) as const:
            # ---- load inputs (split into k-chunks so compute starts early;
            # yT path first: V/K projections lead) ----
            wv_sb = const.tile([128, 8, 256], bf16, tag="wv")
            nc.sync.dma_start(out=wv_sb, in_=wv_d.rearrange("(k p) n -> p k n", p=128))
            wk_sb = const.tile([128, 8, 256], bf16, tag="wk")
            nc.sync.dma_start(out=wk_sb, in_=wk_d.rearrange("(k p) n -> p k n", p=128))
            yT_sb = const.tile([128, 8, S], bf16, tag="yT")
            yT_r = yT_d.rearrange("(k p) t -> p k t", p=128)
            for kk in range(8):
                nc.sync.dma_start(out=yT_sb[:, kk, :], in_=yT_r[:, kk, :])
            cext_sb = const.tile([128, T], f32, tag="cext")
            nc.sync.dma_start(out=cext_sb, in_=cext_d)
            sext_sb = const.tile([128, T], f32, tag="sext")
            nc.sync.dma_start(out=sext_sb, in_=sext_d)
            wq_sb = const.tile([128, 8, 256], bf16, tag="wq")
            nc.sync.dma_start(out=wq_sb, in_=wq_d.rearrange("(k p) n -> p k n", p=128))
            xT_sb = const.tile([128, 8, T], bf16, tag="xT")
            xT_r = xT_d.rearrange("(k p) t -> p k t", p=128)
            for kk in range(8):
                nc.sync.dma_start(out=xT_sb[:, kk, :], in_=xT_r[:, kk, :])
            wo_sb = const.tile([128, 2, DM], bf16, tag="wo")
            nc.sync.dma_start(out=wo_sb, in_=wo_d.rearrange("(i p) n -> p i n", p=128))

            # V with ones column: [128, st, head, 65]
            vsb = const.tile([128, 16, HG, 65], bf16, tag="vsb")
            nc.vector.memset(vsb, 1.0)

            qt = [const.tile([128, T], bf16, tag=f"qt{i}", name=f"qt{i}") for i in range(2)]
            kt = [const.tile([128, S], bf16, tag=f"kt{i}", name=f"kt{i}") for i in range(2)]
            att = [const.tile([128, T], bf16, tag=f"att{i}", name=f"att{i}") for i in range(2)]

            # ---- projections Q, K (with rope) and V ----
            with tc.tile_pool(name="pp", bufs=3, space="PSUM") as pp, \
                 tc.tile_pool(name="pv", bufs=2, space="PSUM") as pvp, \
                 tc.tile_pool(name="ropetmp", bufs=3) as rtp:
                def project(w_sb, act_sb, dst, mt, th):
                    ps = pp.tile([128, 1024], f32, tag="proj", name=f"ps_{dst[0].name}_{mt}_{th}")
                    for c in range(2):
                        for kk in range(8):
                            nc.tensor.matmul(
                                ps[:, ts(c, 512)],
                                lhsT=w_sb[:, kk, ds(mt * 128, 128)],
                                rhs=act_sb[:, kk, ds(th * 1024 + c * 512, 512)],
                                start=(kk == 0),
                                stop=(kk == 7),
                            )
                    tsl = ds(th * 1024, 1024)
                    # rope: roped = qT*cext + shift(qT*sext)
                    nc.vector.tensor_mul(dst[mt][:, tsl], ps, cext_sb[:, tsl])
                    tmp2 = rtp.tile([128, 1024], bf16, tag="tmp2", name=f"t2_{dst[0].name}_{mt}_{th}")
                    nc.vector.tensor_mul(tmp2, ps, sext_sb[:, tsl])
                    tmp2s = rtp.tile([128, 1024], bf16, tag="tmp2s", name=f"t2s_{dst[0].name}_{mt}_{th}")
                    for (do, di, n) in ((0, 16, 16), (16, 0, 16), (32, 32, 32),
                                        (64, 80, 16), (80, 64, 16), (96, 96, 32)):
                        nc.sync.dma_start(out=tmp2s[do:do + n, :], in_=tmp2[di:di + n, :])
                    nc.vector.tensor_add(dst[mt][:, tsl], dst[mt][:, tsl], tmp2s)

                for st in range(16):
                    pv = pvp.tile([128, 256], f32, tag="pv")
                    for kk in range(8):
                        nc.tensor.matmul(
                            pv,
                            lhsT=yT_sb[:, kk, ds(st * 128, 128)],
                            rhs=wv_sb[:, kk, :],
                            start=(kk == 0),
                            stop=(kk == 7),
                        )
                    nc.vector.tensor_copy(
                        vsb[:, st, :, 0:64], pv.rearrange("p (h d) -> p h d", h=HG)
                    )

                # mt0 tiles first so attention hp=0 can begin while mt1 projects
                for mt in range(2):
                    for th in range(2):
                        project(wk_sb, yT_sb, kt, mt, th)
                        project(wq_sb, xT_sb, qt, mt, th)

            # ---- attention ----
            with tc.tile_pool(name="scp", bufs=2, space="PSUM") as scp, \
                 tc.tile_pool(name="avp", bufs=2, space="PSUM") as avp, \
                 tc.tile_pool(name="exl", bufs=8) as exl, \
                 tc.tile_pool(name="nrm", bufs=4) as nrm:
                for hp in range(2):        # head pair tile
                    for th in range(2):    # t halves of 1024
                        avs = [avp.tile([65, 1024], f32, tag="av", name=f"av{hp}_{th}_{i}") for i in range(2)]
                        prev = None

                        def issue_av(prev):
                            st_p, exs_p = prev
                            for i in range(2):
                                for c in range(2):
                                    nc.tensor.matmul(
                                        avs[i][:, ts(c, 512)],
                                        lhsT=vsb[:, st_p, hp * 2 + i, :],
                                        rhs=exs_p[i][:, ts(c, 512)],
                                        start=(st_p == 0),
                                        stop=(st_p == 15),
                                    )

                        for st in range(16):
                            exs = []
                            for i in range(2):  # head within pair
                                ro = i * 64
                                sc = scp.tile([128, 1024], f32, tag="sc")
                                for c in range(2):
                                    nc.tensor.matmul(
                                        sc[:, ts(c, 512)],
                                        lhsT=kt[hp][ro:ro + 64, ds(st * 128, 128)],
                                        rhs=qt[hp][ro:ro + 64, ds(th * 1024 + c * 512, 512)],
                                        start=True,
                                        stop=True,
                                    )
                                ex = exl.tile([128, 1024], bf16, tag="ex")
                                nc.scalar.activation(ex, sc, Exp, scale=0.125)
                                exs.append(ex)
                            if prev is not None:
                                issue_av(prev)
                            prev = (st, exs)
                        issue_av(prev)

                        for i in range(2):
                            ro = i * 64
                            avc = nrm.tile([65, 1024], f32, tag="avc",
                                           name=f"avc{hp}_{th}_{i}")
                            nc.vector.tensor_copy(avc, avs[i])  # frees av psum
                            rec = nrm.tile([1, 1024], f32, tag="rec")
                            nc.vector.reciprocal(rec, avc[64:65, :])
                            bca = nrm.tile([64, 1024], f32, tag="bca")
                            nc.gpsimd.partition_broadcast(bca, rec)
                            nc.vector.tensor_mul(
                                att[hp][ro:ro + 64, ds(th * 1024, 1024)],
                                avc[0:64, :],
                                bca,
                            )

            # ---- output projection ----
            with tc.tile_pool(name="pop", bufs=6, space="PSUM") as pop, \
                 tc.tile_pool(name="osb", bufs=6) as osb:
                for t128 in range(16):
                    for nn in range(2):
                        po = pop.tile([128, 512], f32, tag="po")
                        nc.tensor.matmul(
                            po,
                            lhsT=att[0][:, ds(t128 * 128, 128)],
                            rhs=wo_sb[:, 0, ts(nn, 512)],
                            start=True,
                            stop=False,
                        )
                        nc.tensor.matmul(
                            po,
                            lhsT=att[1][:, ds(t128 * 128, 128)],
                            rhs=wo_sb[:, 1, ts(nn, 512)],
                            start=False,
                            stop=True,
                        )
                        ob = osb.tile([128, 512], f32, tag="ob")
                        if (t128 * 2 + nn) % 2 == 0:
                            nc.vector.tensor_copy(ob, po)
                        else:
                            nc.scalar.copy(ob, po)
                        nc.sync.dma_start(
                            out=out_d[ds(t128 * 128, 128), ts(nn, 512)], in_=ob
                        )

    nc.compile()
    return nc


def _rope_tables():
    """cext/sext [128, T] f32 for the [hd, t] layout (head pairs per tile).

    Rows r (rr = r % 64): rr<32 rope rows, else passthrough.
    cext: cos[t, rr%16] on rope rows, 1.0 on pass rows.
    sext (pre-shifted so tmp2s[r] = tmp2[src(r)], src swaps 16-halves):
      rr<16: +sin[t, rr]; 16<=rr<32: -sin[t, rr-16]; else 0.
    """
    inv_freq = 1.0 / (10000.0 ** (np.arange(0, N_ELEM, 2, dtype=np.float32) / N_ELEM))
    ang = np.arange(T, dtype=np.float32)[:, None] * inv_freq[None, :]
    cosT = np.cos(ang).T.astype(np.float32)  # [16, T]
    sinT = np.sin(ang).T.astype(np.float32)
    cext = np.ones((128, T), np.float32)
    sext = np.zeros((128, T), np.float32)
    for blk in (0, 64):
        for r in range(16):
            cext[blk + r] = cosT[r]
            cext[blk + 16 + r] = cosT[r]
            sext[blk + r] = sinT[r]
            sext[blk + 16 + r] = -sinT[r]
    return cext, sext


def kernel(x, y, cos, sin, mask, Wq, Wk, Wv, Wo):
    import ml_dtypes
    from concourse.bass_utils import run_bass_kernel_spmd

    bf = ml_dtypes.bfloat16
    if "nc" not in _cached:
        _cached["nc"] = _build_program()
    nc = _cached["nc"]

    cext, sext = _rope_tables()
    x = np.asarray(x, dtype=np.float32)
    y = np.asarray(y, dtype=np.float32)
    Wq = np.asarray(Wq, dtype=np.float32)
    Wk = np.asarray(Wk, dtype=np.float32)
    Wv = np.asarray(Wv, dtype=np.float32)
    Wo = np.asarray(Wo, dtype=np.float32)

    in_maps = []
    for c in range(NCORES):
        b, hg = c // 4, c % 4
        cs = slice(hg * 256, (hg + 1) * 256)
        in_maps.append({
            "xT": np.ascontiguousarray(x[b].T).astype(bf),
            "yT": np.ascontiguousarray(y[b].T).astype(bf),
            "wq": np.ascontiguousarray(Wq[:, cs]).astype(bf),
            "wk": np.ascontiguousarray(Wk[:, cs]).astype(bf),
            "wv": np.ascontiguousarray(Wv[:, cs]).astype(bf),
            "wo": np.ascontiguousarray(Wo[cs, :]).astype(bf),
            "cext": cext,
            "sext": sext,
        })

    res = run_bass_kernel_spmd(nc, in_maps, core_ids=list(range(NCORES)))
    parts = [r["out"] for r in res.results]
    out = np.stack([
        parts[0] + parts[1] + parts[2] + parts[3],
        parts[4] + parts[5] + parts[6] + parts[7],
    ]).astype(np.float32)
    return out



# revision 54
# speedup vs baseline: 1.1309x; 1.1309x over previous
"""Cross-attention kernel for Trainium2, 8 NeuronCores.

Problem: B=2, T=S=2048, DM=1024, H=16, HD=64, partial RoPE on first 32 dims.
Sharding: batch (2-way) x head-group (4-way, 4 heads each) = 8 cores.
Each core computes its head-group's contribution to the full output
(out_partial = attn_heads @ Wo_rows); host sums 4 partials per batch.

v2 design (Act-engine-paced):
  - exp is the only Activation-engine work in steady state; it paces the
    attention phase (128 x [128,1024] exp tiles ~ 127us).
  - AV computed transposed: out[t_block=128, hd=64] accumulating over s
    chunks with ex (scores exp, bf16) as the matmul stationary. Halves PE
    rows vs the [65, t] orientation.
  - softmax sum Z via tiny ones-column matmuls (free size 1).
  - normalization: DVE tensor_scalar_mul with per-partition 1/Z.
  - att pairs transposed back to [hd, t] via PE transpose for the output
    projection (contraction over 128 = 2 heads x 64).
  - rope: stream_shuffle (DVE partition shuffle in 32-groups) instead of
    SBUF shift DMAs.
  - projections JIT-interleaved into the attention stream: only K/Q half-0
    run before head 0; V proj runs inside head 0's loop, K/Q half-1 inside
    head 1's loop, in <=4-matmul chunks to avoid starving the Act engine.
    PSUM: scores(4) + av(2) + z(1) banks + 1 bank time-shared by pv/proj/
    transpose pools.
  - phase 3: 4 big output DMAs, PSUM->SBUF copies split across Act+DVE.
"""

import numpy as np

B, T, S, DM = 2, 2048, 2048, 1024
H, HD, N_ELEM = 16, 64, 32
HG = 4          # heads per core
NCORES = 8

_cached = {}


def _build_program():
    import concourse.bass as bass
    import concourse.tile as tile
    from concourse import bacc, mybir
    from concourse.bass import ts, ds

    f32 = mybir.dt.float32
    bf16 = mybir.dt.bfloat16
    Exp = mybir.ActivationFunctionType.Exp

    nc = bacc.Bacc(
        "TRN2",
        target_bir_lowering=False,
        debug=False,
        enable_asserts=False,
        num_devices=NCORES,
    )

    xT_d = nc.dram_tensor("xT", [DM, T], bf16, kind="ExternalInput").ap()
    yT_d = nc.dram_tensor("yT", [DM, S], bf16, kind="ExternalInput").ap()
    wq_d = nc.dram_tensor("wq", [DM, 256], bf16, kind="ExternalInput").ap()
    wk_d = nc.dram_tensor("wk", [DM, 256], bf16, kind="ExternalInput").ap()
    wv_d = nc.dram_tensor("wv", [DM, 256], bf16, kind="ExternalInput").ap()
    wo_d = nc.dram_tensor("wo", [256, DM], bf16, kind="ExternalInput").ap()
    cext_d = nc.dram_tensor("cext", [128, T], bf16, kind="ExternalInput").ap()
    sext_d = nc.dram_tensor("sext", [128, T], bf16, kind="ExternalInput").ap()
    out_d = nc.dram_tensor("out", [T, DM], bf16, kind="ExternalOutput").ap()

    SHIFT16 = [(i + 16) % 32 for i in range(32)]

    with tile.TileContext(nc) as tc:
        with tc.tile_pool(name="const", bufs=1) as const:
            # ---- load inputs, ordered for earliest first score ----
            wk_sb = const.tile([128, 8, 256], bf16, tag="wk")
            nc.sync.dma_start(out=wk_sb, in_=wk_d.rearrange("(k p) n -> p k n", p=128))
            wq_sb = const.tile([128, 8, 256], bf16, tag="wq")
            nc.sync.dma_start(out=wq_sb, in_=wq_d.rearrange("(k p) n -> p k n", p=128))

            xT_sb = const.tile([128, 8, T], bf16, tag="xT")
            xT_r = xT_d.rearrange("(k p) t -> p k t", p=128)
            yT_sb = const.tile([128, 8, S], bf16, tag="yT")
            yT_r = yT_d.rearrange("(k p) t -> p k t", p=128)
            cext_sb = const.tile([128, T], bf16, tag="cext")
            sext_sb = const.tile([128, T], bf16, tag="sext")
            wv_sb = const.tile([128, 8, 256], bf16, tag="wv")
            wo_sb = const.tile([128, 2, DM], bf16, tag="wo")

            # critical set for the first scores first, channel-split
            for ch in range(2):
                cs = ds(ch * 1024, 1024)
                nc.sync.dma_start(out=yT_sb[:, :, cs], in_=yT_r[:, :, cs])
                nc.sync.dma_start(out=cext_sb[:, cs], in_=cext_d[:, cs])
                nc.sync.dma_start(out=sext_sb[:, cs], in_=sext_d[:, cs])
                nc.sync.dma_start(out=xT_sb[:, :, cs], in_=xT_r[:, :, cs])
            # identity matrix for PE transpose: ident[p, f] = (f == p)
            colv = const.tile([128, 128], f32, tag="colv")
            nc.gpsimd.iota(colv, pattern=[[1, 128]], base=0, channel_multiplier=0,
                           allow_small_or_imprecise_dtypes=True)
            rowv = const.tile([128, 1], f32, tag="rowv")
            nc.gpsimd.iota(rowv, pattern=[[0, 1]], base=0, channel_multiplier=1,
                           allow_small_or_imprecise_dtypes=True)
            ident = const.tile([128, 128], bf16, tag="ident")
            nc.vector.tensor_scalar(
                ident, colv, rowv, None, mybir.AluOpType.is_equal
            )
            ones_sb = const.tile([128, 1], bf16, tag="ones")
            nc.vector.memset(ones_sb, 1.0)
            # preload the exp table at t~0 so LoadActFuncSet is off the
            # critical path (it otherwise runs right before the first real exp)
            dummy = const.tile([1, 1], f32, tag="dummy")
            nc.vector.memset(dummy, 0.0)
            nc.scalar.activation(dummy, dummy, Exp, scale=1.0)

            # persistent SBUF tensors
            kt = [const.tile([128, S], bf16, tag=f"kt{i}", name=f"kt{i}") for i in range(2)]
            qt = [const.tile([128, T], bf16, tag=f"qt{i}", name=f"qt{i}") for i in range(2)]
            vsb = const.tile([128, 16, HG, 64], bf16, tag="vsb")
            # att (post-transpose, [hd-pair, t]) in 4 t-groups per pair for
            # fine-grained phase-3 deps
            attp = [[const.tile([128, 512], bf16, tag=f"attp{i}_{g}",
                                name=f"attp{i}_{g}") for g in range(4)]
                    for i in range(2)]

            with tc.tile_pool(name="ropetmp", bufs=2) as rtp:

                def rope_start(dst, mt, tsl_base, width, ps):
                    """dst[:, tsl] = ps*cext; t2 = shuffle16(ps)*sext on
                    GPSIMD (otherwise idle)."""
                    tsl = ds(tsl_base, width)
                    shf = rtp.tile([128, width], f32, tag=f"shf{width}",
                                   name=f"shf_{dst[0].name}_{mt}_{tsl_base}")
                    nc.vector.stream_shuffle(shf, ps, SHIFT16)
                    nc.vector.tensor_mul(dst[mt][:, tsl], ps, cext_sb[:, tsl])
                    t2 = rtp.tile([128, width], bf16, tag=f"t2{width}",
                                  name=f"t2_{dst[0].name}_{mt}_{tsl_base}")
                    nc.gpsimd.tensor_mul(t2, shf, sext_sb[:, tsl])
                    return (dst, mt, tsl, t2)

                def rope_end(rs):
                    dst, mt, tsl, t2 = rs
                    nc.vector.tensor_add(dst[mt][:, tsl], dst[mt][:, tsl], t2)

                def rope_fin(dst, mt, tsl_base, width, ps):
                    rope_end(rope_start(dst, mt, tsl_base, width, ps))

                # ---- prologue: K/Q half-0 projections (own psum pool) ----
                with tc.tile_pool(name="pp0", bufs=2, space="PSUM") as pp0:
                    # PE p-state warmup: dep-free matmuls during the input
                    # DMAs so the PE clock is at full speed for the real work
                    wmt = const.tile([128, 512], bf16, tag="wm")
                    nc.vector.memset(wmt, 0.0)
                    wps = pp0.tile([128, 512], f32, tag="warm")
                    for i in range(16):
                        nc.tensor.matmul(
                            wps, lhsT=wmt[:, 0:128], rhs=wmt,
                            start=True, stop=True,
                        )
                    # th0 for both K and Q first: the first score tile only
                    # needs kt/qt cols 0:1024, so its rope chain is 2 tiles;
                    # rope start/end split so the two tiles' ropes overlap
                    for th in range(2):
                        rss = []
                        for w_sb, act_sb, dst in ((wk_sb, yT_sb, kt),
                                                  (wq_sb, xT_sb, qt)):
                            ps = pp0.tile([128, 1024], f32, tag="proj",
                                          name=f"ps0_{dst[0].name}_{th}")
                            for c in range(2):
                                for kk in range(8):
                                    nc.tensor.matmul(
                                        ps[:, ts(c, 512)],
                                        lhsT=w_sb[:, kk, ds(0, 128)],
                                        rhs=act_sb[:, kk,
                                                   ds(th * 1024 + c * 512, 512)],
                                        start=(kk == 0),
                                        stop=(kk == 7),
                                    )
                            rss.append(rope_start(dst, 0, th * 1024, 1024, ps))
                        for rs in rss:
                            rope_end(rs)

                # V-projection + out-proj weights: emitted after the
                # prologue matmuls so they stay out of the PE wait-merge
                nc.sync.dma_start(out=wv_sb,
                                  in_=wv_d.rearrange("(k p) n -> p k n", p=128))
                nc.sync.dma_start(out=wo_sb,
                                  in_=wo_d.rearrange("(i p) n -> p i n", p=128))

                # ---- attention (Act-paced; sequential heads) ----
                avp = tc.alloc_tile_pool(name="avp", bufs=1, space="PSUM")
                zp = tc.alloc_tile_pool(name="zp", bufs=1, space="PSUM")
                scr = tc.alloc_tile_pool(name="scr", bufs=1, space="PSUM")
                scp = tc.alloc_tile_pool(name="scp", bufs=2, space="PSUM")
                exl = tc.alloc_tile_pool(name="exl", bufs=3)
                asbp = tc.alloc_tile_pool(name="asb", bufs=2)
                rcp = tc.alloc_tile_pool(name="rcp", bufs=2)
                if True:

                    # V projection chunks (h0): 4+4 matmuls into 1-bank psum
                    pv_cur = [None]

                    def vproj_chunk(st, half):
                        if half == 0:
                            pv_cur[0] = scr.tile([128, 256], f32, tag="scr",
                                                 name=f"pv{st}")
                        pv = pv_cur[0]
                        for kk in range(4 * half, 4 * half + 4):
                            nc.tensor.matmul(
                                pv,
                                lhsT=yT_sb[:, kk, ds(st * 128, 128)],
                                rhs=wv_sb[:, kk, :],
                                start=(kk == 0),
                                stop=(kk == 7),
                            )
                        if half == 1:
                            nc.vector.tensor_copy(
                                vsb[:, st, :, :],
                                pv.rearrange("p (h d) -> p h d", h=HG),
                            )

                    def kq1_chunks():
                        """Closures: each one 256-wide DR projection chunk
                        (4 matmuls) plus its rope; 1KB psum tiles so two can
                        pipeline in the shared 1-bank scratch pool."""
                        out = []
                        for w_sb, act_sb, dst in ((wk_sb, yT_sb, kt),
                                                  (wq_sb, xT_sb, qt)):
                            for th in range(2):
                                for c in range(4):
                                    def chunk(w_sb=w_sb, act_sb=act_sb,
                                              dst=dst, th=th, c=c):
                                        base = th * 1024 + c * 256
                                        ps = scr.tile(
                                            [128, 256], f32, tag="scr",
                                            name=f"ps1_{dst[0].name}_{th}_{c}")
                                        for kk in range(8):
                                            nc.tensor.matmul(
                                                ps,
                                                lhsT=w_sb[:, kk, ds(128, 128)],
                                                rhs=act_sb[:, kk, ds(base, 256)],
                                                start=(kk == 0),
                                                stop=(kk == 7),
                                            )
                                        rope_fin(dst, 1, base, 256, ps)
                                    out.append(chunk)
                        return out

                    deferred = []

                    pair_sb = None
                    for h in range(HG):
                        hp, ro = h // 2, (h % 2) * 64
                        if h == 1:
                            deferred = kq1_chunks()
                        av = avp.tile([128, 16, 64], f32, tag="av", name=f"av{h}")
                        zt = zp.tile([128, 16], f32, tag="z", name=f"z{h}")
                        # a start=True matmul zeroes its whole 2KB PSUM bank,
                        # so interleaved per-tb accumulators must pre-zero via
                        # memset and accumulate with start=False throughout
                        nc.vector.memset(av, 0.0)
                        nc.vector.memset(zt, 0.0)
                        if h % 2 == 0:
                            pair_sb = asbp.tile([128, 16, 128], bf16, tag="pair",
                                                name=f"pair{hp}")

                        def issue_av(st_p, ex_p, h=h, av=av, zt=zt):
                            for tb in range(16):
                                nc.tensor.matmul(
                                    av[:, tb, :],
                                    lhsT=ex_p[:, ds(tb * 128, 128)],
                                    rhs=vsb[:, st_p, h, :],
                                    start=False,
                                    stop=(st_p == 15),
                                    skip_group_check=True,
                                )
                                nc.tensor.matmul(
                                    zt[:, tb:tb + 1],
                                    lhsT=ex_p[:, ds(tb * 128, 128)],
                                    rhs=ones_sb,
                                    start=False,
                                    stop=(st_p == 15),
                                    skip_group_check=True,
                                )

                        prev = None
                        for st in range(16):
                            ex = exl.tile([128, 2048], bf16, tag="ex",
                                          name=f"ex{h}_{st}")
                            for th in range(2):
                                sc = scp.tile([128, 1024], f32, tag="sc")
                                for c in range(2):
                                    nc.tensor.matmul(
                                        sc[:, ts(c, 512)],
                                        lhsT=kt[hp][ro:ro + 64, ds(st * 128, 128)],
                                        rhs=qt[hp][ro:ro + 64,
                                                   ds(th * 1024 + c * 512, 512)],
                                        start=True,
                                        stop=True,
                                    )
                                nc.scalar.activation(
                                    ex[:, ds(th * 1024, 1024)], sc, Exp,
                                    scale=0.125,
                                )
                                # JIT-interleaved projection work
                                if h == 0:
                                    vproj_chunk(st, th)
                                elif deferred:
                                    deferred.pop(0)()
                            if prev is not None:
                                issue_av(*prev)
                            prev = (st, ex)
                        issue_av(*prev)
                        while deferred:
                            deferred.pop(0)()
                        if h == 3:
                            scp.release()   # free 4 banks for out-proj psum

                        # normalize: att[t, hd] = av[t, hd] / Z[t]
                        rec = rcp.tile([128, 16], f32, tag="rec", name=f"rec{h}")
                        nc.vector.reciprocal(rec, zt)
                        if h < 3:
                            for tb in range(16):
                                nc.vector.tensor_scalar_mul(
                                    pair_sb[:, tb, ds(ro, 64)], av[:, tb, :],
                                    rec[:, tb:tb + 1],
                                )
                        # pair 0 complete: transpose [t,128]->[128,t]
                        if h == 1:
                            for tg in range(4):
                                for ti in range(4):
                                    tp_t = scr.tile([128, 128], bf16,
                                                    tag="scr")
                                    nc.tensor.transpose(
                                        tp_t, pair_sb[:, tg * 4 + ti, :],
                                        ident,
                                    )
                                    nc.vector.tensor_copy(
                                        attp[0][tg][:, ds(ti * 128, 128)],
                                        tp_t,
                                    )

                    # ---- fused tail for head 3: per t-group, pipeline
                    # norm -> transpose -> out-proj -> DMA ----
                    out_r = out_d.rearrange("(a p) n -> p a n", p=128)
                    pop = tc.alloc_tile_pool(name="pop", bufs=4, space="PSUM")
                    osb = tc.alloc_tile_pool(name="osb", bufs=2)
                    for tg in range(4):
                        for tb in range(tg * 4, tg * 4 + 4):
                            nc.vector.tensor_scalar_mul(
                                pair_sb[:, tb, ds(64, 64)], av[:, tb, :],
                                rec[:, tb:tb + 1],
                            )
                        for ti in range(4):
                            tp_t = scr.tile([128, 128], bf16, tag="scr")
                            nc.tensor.transpose(
                                tp_t, pair_sb[:, tg * 4 + ti, :], ident,
                            )
                            nc.vector.tensor_copy(
                                attp[1][tg][:, ds(ti * 128, 128)], tp_t,
                            )
                        obg = osb.tile([128, 4, 1024], bf16, tag="obg")
                        for t4 in range(4):
                            for nn in range(2):
                                po = pop.tile([128, 512], f32, tag="po")
                                nc.tensor.matmul(
                                    po,
                                    lhsT=attp[0][tg][:, ds(t4 * 128, 128)],
                                    rhs=wo_sb[:, 0, ts(nn, 512)],
                                    start=True,
                                    stop=False,
                                )
                                nc.tensor.matmul(
                                    po,
                                    lhsT=attp[1][tg][:, ds(t4 * 128, 128)],
                                    rhs=wo_sb[:, 1, ts(nn, 512)],
                                    start=False,
                                    stop=True,
                                )
                                if nn == 0:
                                    nc.vector.tensor_copy(
                                        obg[:, t4, ts(nn, 512)], po)
                                else:
                                    nc.scalar.copy(obg[:, t4, ts(nn, 512)], po)
                        nc.sync.dma_start(
                            out=out_r[:, ds(tg * 4, 4), :], in_=obg
                        )
                    pop.release()
                    scr.release()
                    zp.release()
                    osb.release()
                    scp = None
                for p in (rcp, asbp, exl, avp):
                    p.release()

    nc.compile()
    return nc


def _rope_tables():
    """cext/sext [128, T] f32 for the [hd, t] layout (head pairs per tile).

    Rows r (rr = r % 64): rr<32 rope rows, else passthrough.
    cext: cos[t, rr%16] on rope rows, 1.0 on pass rows.
    sext is multiplied at the DEST row after the 16-shift (shf[r] =
    ps[(r+16)%32 within the 32-group]):
      rr<16: -sin[t, rr]; 16<=rr<32: +sin[t, rr-16]; else 0.
    """
    inv_freq = 1.0 / (10000.0 ** (np.arange(0, N_ELEM, 2, dtype=np.float32) / N_ELEM))
    ang = np.arange(T, dtype=np.float32)[:, None] * inv_freq[None, :]
    cosT = np.cos(ang).T.astype(np.float32)  # [16, T]
    sinT = np.sin(ang).T.astype(np.float32)
    cext = np.ones((128, T), np.float32)
    sext = np.zeros((128, T), np.float32)
    for blk in (0, 64):
        for r in range(16):
            cext[blk + r] = cosT[r]
            cext[blk + 16 + r] = cosT[r]
            sext[blk + r] = -sinT[r]
            sext[blk + 16 + r] = sinT[r]
    return cext, sext


def _make_in_maps(x, y, Wq, Wk, Wv, Wo):
    import ml_dtypes

    bf = ml_dtypes.bfloat16
    cext, sext = _rope_tables()
    cext = cext.astype(bf)
    sext = sext.astype(bf)
    x = np.asarray(x, dtype=np.float32)
    y = np.asarray(y, dtype=np.float32)
    Wq = np.asarray(Wq, dtype=np.float32)
    Wk = np.asarray(Wk, dtype=np.float32)
    Wv = np.asarray(Wv, dtype=np.float32)
    Wo = np.asarray(Wo, dtype=np.float32)

    in_maps = []
    for c in range(NCORES):
        b, hg = c // 4, c % 4
        cs = slice(hg * 256, (hg + 1) * 256)
        in_maps.append({
            "xT": np.ascontiguousarray(x[b].T).astype(bf),
            "yT": np.ascontiguousarray(y[b].T).astype(bf),
            "wq": np.ascontiguousarray(Wq[:, cs]).astype(bf),
            "wk": np.ascontiguousarray(Wk[:, cs]).astype(bf),
            "wv": np.ascontiguousarray(Wv[:, cs]).astype(bf),
            "wo": np.ascontiguousarray(Wo[cs, :]).astype(bf),
            "cext": cext,
            "sext": sext,
        })
    return in_maps


def kernel(x, y, cos, sin, mask, Wq, Wk, Wv, Wo):
    from concourse.bass_utils import run_bass_kernel_spmd

    if "nc" not in _cached:
        _cached["nc"] = _build_program()
    nc = _cached["nc"]

    in_maps = _make_in_maps(x, y, Wq, Wk, Wv, Wo)
    res = run_bass_kernel_spmd(nc, in_maps, core_ids=list(range(NCORES)))
    parts = [np.asarray(r["out"], dtype=np.float32) for r in res.results]
    out = np.stack([
        parts[0] + parts[1] + parts[2] + parts[3],
        parts[4] + parts[5] + parts[6] + parts[7],
    ]).astype(np.float32)
    return out
